# revision 35
# baseline (speedup 1.0000x reference)
"""Multi-head causal attention (B=4, T=2048, N=1024, H=16) on 8 TRN2 NeuronCores.

Sharding: core c = (batch b = c//2, head-group hg = c%2). Each core computes
full-T causal attention for its 8 heads of its batch, plus the partial output
projection for its head rows. Host sums the two head-group partials per batch
and adds b_proj (and the v-bias contribution, which is exact because softmax
rows sum to 1).

v3 schedule — one fused stream balancing PE against the ACT engine (exp):
  - Work order: qkv(0); att(0) [qkv(1) bg]; att(1) [qkv(3) bg]; then per
    head-pair: att(3) full + att(2) kb0..7 spilled to SBUF [qkv(2), proj(0),
    proj(1) bg]; att(2) kb8..11 resumed + normalized [proj(3) bg]; proj(2).
    This spreads attention exp (ACT-bound, 2:1 vs its PE stream) across the
    qkv/proj filler so neither engine stalls long.
  - Scores per (head-pair, k-block) are two K=64 matmuls on PE row groups
    0/64 (run concurrently); emitted two k-blocks at a time so the PE
    64/128-row tiling mode switches half as often.
  - ACT runs ONLY exp: one [128,1024] activation per k-block spanning both
    heads' score banks. Bias-adds / drains / normalize are DVE+GPSIMD.
  - Projection in bf16 (weights + normalized y) to shorten the exposed tail.
PSUM: score-pair tile (2 banks) x2 + y accumulators x2 + qkv/proj rotation x2.
"""
import numpy as np
import ml_dtypes
from contextlib import ExitStack

import concourse.bass as bass
import concourse.mybir as mybir
from concourse import bacc
from concourse import bass_utils as _bu
from concourse.bass_utils import run_bass_kernel_spmd
from concourse.tile import TileContext



F32 = mybir.dt.float32
F32R = mybir.dt.float32r
BF16 = mybir.dt.bfloat16
AF = mybir.ActivationFunctionType

B, T, N, H = 4, 2048, 1024, 16
Dh = 64
HG = 512            # head-group width per core (8 heads x 64)
NCORES = 8
KT = N // 128       # 8 contraction tiles for qkv
NQC = T // 512      # 4 q-chunks
NVT = T // 128      # 16 v tiles / k blocks

_CACHE = {}
_last_results = None


def _build():
    if "nc" in _CACHE:
        return _CACHE["nc"]

    nc = bacc.Bacc("TRN2", target_bir_lowering=False)

    # all inputs pre-transposed host-side to [128 partitions, ...] so each
    # DMA descriptor covers the full per-partition extent (contiguous)
    xT = nc.declare_dram_parameter("xT", [128, 4, KT, 512], BF16,
                                   isOutput=False)
    wqk = nc.declare_dram_parameter("wqk", [128, KT, 2 * HG], BF16,
                                    isOutput=False)
    wv = nc.declare_dram_parameter("wv", [128, KT, HG], BF16, isOutput=False)
    wp = nc.declare_dram_parameter("wp", [128, 4, N], BF16, isOutput=False)
    bqk = nc.declare_dram_parameter("bqk", [128, 8, 1], F32, isOutput=False)
    maskt = nc.declare_dram_parameter("maskt", [128, 4, 1024], BF16,
                                      isOutput=False)
    # bias-form masks (0 keep / -240 masked) + identity in slice 4 cols 0:128
    maskb = nc.declare_dram_parameter("maskb", [128, 5, 1024], BF16,
                                      isOutput=False)
    out = nc.declare_dram_parameter("out", [T, N], F32, isOutput=True)

    with TileContext(nc) as tc, ExitStack() as outer:
        consts = outer.enter_context(tc.tile_pool(name="consts", bufs=1))
        resid = outer.enter_context(tc.tile_pool(name="resid", bufs=1))
        wpool = outer.enter_context(tc.tile_pool(name="wpool", bufs=1))
        xpool = outer.enter_context(tc.tile_pool(name="xpool", bufs=3))
        attTp = outer.enter_context(tc.tile_pool(name="attTp", bufs=5))
        smallp = outer.enter_context(tc.tile_pool(name="smallp", bufs=2))
        bcp = outer.enter_context(tc.tile_pool(name="bcp", bufs=2))
        spillp = outer.enter_context(tc.tile_pool(name="spillp", bufs=1))
        ytp = outer.enter_context(tc.tile_pool(name="ytp", bufs=3))
        outp = outer.enter_context(tc.tile_pool(name="outp", bufs=2))
        # PSUM pools: 2*2 + 2*1 + 2*1 = 8 banks
        sps = outer.enter_context(tc.tile_pool(name="sps", bufs=2,
                                               space="PSUM"))
        yps = outer.enter_context(tc.tile_pool(name="yps", bufs=2,
                                               space="PSUM"))
        gps = outer.enter_context(tc.tile_pool(name="gps", bufs=2,
                                               space="PSUM"))

        # ---- DMAs: one consolidated trigger per tensor group ----
        # (each DRAM view is transposed into [128, k, cols] tiles so a
        # single descriptor-rich DMA replaces 8 separate triggers)
        xt_chunks = [None] * 4

        def fetch_xt(tck):
            t = xpool.tile([128, KT, 512], BF16, name=f"xt{tck}", tag="xt")
            nc.sync.dma_start(out=t, in_=xT[:, tck, :, :])
            xt_chunks[tck] = t

        # startup triggers split across both HWDGE queues (scalar queue is
        # exp-free until ~25us, so a few early triggers are harmless there)
        bqk_sb = consts.tile([128, 8, 1], F32, name="bqk_sb", tag="bqk")
        nc.scalar.dma_start(out=bqk_sb, in_=bqk[:, :, :])
        fetch_xt(0)
        wv_sb = wpool.tile([128, KT, HG], BF16, name="wv_sb", tag="wv")
        nc.scalar.dma_start(out=wv_sb, in_=wv[:, :, :])
        wqk_sb = wpool.tile([128, KT, 2 * HG], BF16, name="wqk_sb",
                            tag="wqk")
        nc.scalar.dma_start(out=wqk_sb, in_=wqk[:, :, :])
        mt_all = consts.tile([128, 4, 1024], BF16, name="mt_all", tag="mt")
        nc.sync.dma_start(out=mt_all, in_=maskt[:, :, :])
        mb_all = consts.tile([128, 5, 1024], BF16, name="mb_all", tag="mb")
        nc.sync.dma_start(out=mb_all, in_=maskb[:, :, :])
        fetch_xt(1)
        wp_sb = wpool.tile([128, 4, N], BF16, name="wp_sb", tag="wp")
        nc.scalar.dma_start(out=wp_sb, in_=wp[:, :, :])
        fetch_xt(2)

        wv_t = [wv_sb[:, k, :] for k in range(KT)]
        wqk_t = [wqk_sb[:, k, :] for k in range(KT)]
        wp_t = [wp_sb[:, j, :] for j in range(4)]
        bqk_sb = [bqk_sb[:, j, :] for j in range(8)]
        mt_sb = [mt_all[:, m, :] for m in range(4)]
        mb_sb = [mb_all[:, m, :] for m in range(4)]
        ident = mb_all[:, 4, 0:128]

        # residents: qT/kT per head-pair tile [128, T]; v per 128-row chunk,
        # bf16, layout [64 v-cols + ones] x 8 heads = 520 cols
        qT = [resid.tile([128, T], BF16, name=f"qT{j}", tag=f"qT{j}")
              for j in range(4)]
        kTt = [resid.tile([128, T], BF16, name=f"kT{j}", tag=f"kT{j}")
               for j in range(4)]
        vt = [resid.tile([128, 8, 65], BF16, name=f"v{m}", tag=f"v{m}")
              for m in range(NVT)]
        for m in range(NVT):
            nc.gpsimd.memset(vt[m][:, :, 64:65], 1.0)

        yt_all = {}     # qc -> [4] yt tiles (bf16, normalized yT)
        spill = {}      # (hp, h01) -> [65, 512] f32 SBUF partial (qc=2)

        class Bg:
            """FIFO of background PE micro-ops sprinkled between units."""
            def __init__(self, ops=()):
                self.ops = list(ops)
                self.done = 0

            def add(self, ops):
                self.ops.extend(ops)

            def pop(self, n=1):
                for _ in range(min(n, len(self.ops))):
                    self.ops.pop(0)()
                    self.done += 1

            def drain_to(self, k):
                while self.done < k and self.ops:
                    self.pop()

            def drain(self):
                while self.ops:
                    self.pop()

            def __len__(self):
                return len(self.ops)

        # ---- background micro-op machinery ----
        def qkv_ops(tck, drain_eng):
            """Returns {'v': [36 ops], 'pairs': [[18 ops] x 4 head-pairs]}."""
            c0 = tck * 512
            xt = xt_chunks[tck]

            def add_chain(ops, mk_mm, mk_drain):
                ps_ref = []
                for k in range(KT):
                    ops.append(mk_mm(k, ps_ref))
                ops.append(mk_drain(ps_ref))

            v_ops = []
            for mc in range(4):
                def mk_mm(k, ps_ref, mc=mc):
                    def go():
                        if k == 0:
                            ps_ref.append(gps.tile(
                                [128, HG], F32, name=f"v_ps{tck}_{mc}",
                                tag="gp"))
                        nc.tensor.matmul(
                            ps_ref[0],
                            xt[:, k, mc * 128:(mc + 1) * 128],
                            wv_t[k],
                            start=(k == 0), stop=(k == KT - 1))
                    return go

                def mk_drain(ps_ref, mc=mc):
                    def go():
                        dst = vt[tck * 4 + mc]
                        src = ps_ref[0].rearrange("p (h c) -> p h c", c=64)
                        if drain_eng is nc.scalar:
                            nc.scalar.copy(dst[:, :, 0:64], src)
                        else:
                            drain_eng.tensor_copy(dst[:, :, 0:64], src)
                    return go
                add_chain(v_ops, mk_mm, mk_drain)

            pair_ops = []
            for hp in range(4):
                ops = []
                for jc in (hp, hp + 4):
                    def mk_mm(k, ps_ref, jc=jc):
                        def go():
                            if k == 0:
                                ps_ref.append(gps.tile(
                                    [128, 512], F32,
                                    name=f"qk_ps{tck}_{jc}", tag="gp"))
                            nc.tensor.matmul(
                                ps_ref[0],
                                wqk_t[k][:, jc * 128:(jc + 1) * 128],
                                xt[:, k, :],
                                start=(k == 0), stop=(k == KT - 1))
                        return go

                    def mk_drain(ps_ref, jc=jc):
                        def go():
                            dst = (qT[jc] if jc < 4 else kTt[jc - 4])
                            if drain_eng is nc.scalar:
                                nc.scalar.add(dst[:, c0:c0 + 512],
                                              ps_ref[0], bqk_sb[jc])
                            else:
                                drain_eng.tensor_scalar_add(
                                    dst[:, c0:c0 + 512], ps_ref[0],
                                    bqk_sb[jc])
                        return go
                    add_chain(ops, mk_mm, mk_drain)
                pair_ops.append(ops)
            return {"v": v_ops, "pairs": pair_ops}

        def qkv_flat(q):
            ops = list(q["v"])
            for p in q["pairs"]:
                ops.extend(p)
            return ops

        def proj_ops(qc, drain_eng, hp_order=(0, 1, 2, 3)):
            qcol = qc * 512
            yt_sb = yt_all[qc]
            ops = []
            for qb in range(4):
                o_ref = []
                for nch in range(2):
                    ps_ref = []

                    def mk_mm(i, qb=qb, nch=nch, ps_ref=ps_ref,
                              o_ref=o_ref):
                        hp = hp_order[i]

                        def go():
                            if i == 0:
                                ps_ref.append(gps.tile(
                                    [128, 512], F32,
                                    name=f"p{qc}_{qb}_{nch}", tag="gp"))
                            if nch == 0 and i == 0:
                                o_ref.append(outp.tile(
                                    [128, N], F32, name=f"o{qc}_{qb}",
                                    tag="o"))
                            nc.tensor.matmul(
                                ps_ref[0],
                                yt_sb[hp][:, qb * 128:(qb + 1) * 128],
                                wp_t[hp][:, nch * 512:(nch + 1) * 512],
                                start=(i == 0), stop=(i == 3))
                        return go
                    for i in range(4):
                        ops.append(mk_mm(i, qb, nch, ps_ref, o_ref))

                    def mk_drain(qb=qb, nch=nch, ps_ref=ps_ref,
                                 o_ref=o_ref):
                        def go():
                            drain_eng.tensor_copy(
                                o_ref[0][:, nch * 512:(nch + 1) * 512],
                                ps_ref[0])
                            if nch == 1:
                                nc.sync.dma_start(
                                    out=out[qcol + qb * 128:
                                            qcol + (qb + 1) * 128, :],
                                    in_=o_ref[0])
                        return go
                    ops.append(mk_drain(qb, nch, ps_ref, o_ref))
            return ops

        # ---- attention block: one (qc, hp), k-blocks [kb_lo, kb_hi) ----
        # pops ceil(bg_share) background ops per pair-batch. mask_pe=True
        # applies the causal mask as an identity-matmul bias into the score
        # PSUM before exp (PE) instead of a post-exp DVE multiply.
        def att_block(qc, hp, kb_lo, kb_hi, mode, bg, batches_after,
                      mask_pe=False):
            qcol = qc * 512
            dlo = 4 * qc                   # first diagonal k-block
            y_ps = [yps.tile([65, 512], F32,
                             name=f"y{qc}_{hp}_{kb_lo}_{h01}", tag="y")
                    for h01 in range(2)]

            def emit_avs(at_l):
                for attT, kb in at_l:
                    for h01 in range(2):
                        nc.tensor.matmul(
                            y_ps[h01],
                            vt[kb][:, hp * 2 + h01, :],
                            attT[:, h01 * 512:(h01 + 1) * 512],
                            start=(kb == kb_lo),
                            stop=(kb == kb_hi - 1))

            pend = None
            nbatch = (kb_hi - kb_lo) // 2
            for bi in range(nbatch):
                kb0 = kb_lo + 2 * bi
                at_l = []
                for kb in (kb0, kb0 + 1):
                    diag = kb >= dlo
                    s_ps = sps.tile([128, 1024], F32,
                                    name=f"s{qc}_{hp}_{kb}", tag="s")
                    for h01 in range(2):
                        hb = h01 * 64
                        nc.tensor.matmul(
                            s_ps[:, h01 * 512:(h01 + 1) * 512],
                            kTt[hp][hb:hb + 64,
                                    kb * 128:(kb + 1) * 128],
                            qT[hp][hb:hb + 64, qcol:qcol + 512],
                            start=True, stop=not (diag and mask_pe))
                    if diag and mask_pe:
                        for h01 in range(2):
                            nc.tensor.matmul(
                                s_ps[:, h01 * 512:(h01 + 1) * 512],
                                ident,
                                mb_sb[kb - dlo][:, h01 * 512:
                                                (h01 + 1) * 512],
                                start=False, stop=True)
                    attT = attTp.tile([128, 1024], BF16,
                                      name=f"attT{qc}_{hp}_{kb}",
                                      tag="attT")
                    nc.scalar.activation(out=attT, in_=s_ps,
                                         func=AF.Exp, scale=0.125)
                    if diag and not mask_pe:
                        nc.vector.tensor_mul(attT, attT, mt_sb[kb - dlo])
                    at_l.append((attT, kb))
                if pend is not None:
                    emit_avs(pend)
                pend = at_l
                nleft = batches_after + (nbatch - 1 - bi)
                npop = -(-len(bg) // (nleft + 1))
                bg.pop(npop)
            emit_avs(pend)

            for h01 in range(2):
                hb = h01 * 64
                if mode == "spill":
                    sp = spillp.tile([65, 512], F32,
                                     name=f"sp{qc}_{hp}_{h01}",
                                     tag=f"sp{qc}{hp}{h01}")
                    nc.vector.tensor_copy(sp, y_ps[h01])
                    spill[(qc, hp, h01)] = sp
                    continue
                if mode == "final_spill":
                    # src lands in SBUF, so the copy/scale can use GPSIMD
                    sp = spill[(qc, hp, h01)]
                    nc.vector.tensor_add(sp, sp, y_ps[h01])
                    src = sp
                    copy_eng = mul_eng = nc.vector
                else:
                    src = y_ps[h01]
                    copy_eng = mul_eng = nc.vector
                srow = smallp.tile([1, 512], F32,
                                   name=f"srow{qc}_{hp}_{h01}", tag="srow")
                copy_eng.tensor_copy(srow, src[64:65, :])
                nc.vector.reciprocal_approx_fast(out=srow, in_=srow)
                bc = bcp.tile([64, 512], F32,
                              name=f"bc{qc}_{hp}_{h01}", tag="bc")
                nc.gpsimd.partition_broadcast(bc, srow)
                mul_eng.tensor_mul(yt_all[qc][hp][hb:hb + 64, :],
                                   src[0:64, :], bc)

        def new_yt(qc):
            yt_all[qc] = [ytp.tile([128, 512], BF16, name=f"yt{qc}_{hp}",
                                   tag=f"yt{hp}") for hp in range(4)]

        # ---- the fused stream ----
        # R01: qkv(0) v-chains + first qk pair direct, then att(0)+att(1)
        # with the rest of qkv(0) and qkv(1) as background.
        q0 = qkv_ops(0, nc.vector)
        for op in q0["v"] + q0["pairs"][0]:
            op()
        new_yt(0)
        new_yt(1)
        q1 = qkv_ops(1, nc.vector)
        bg = Bg(q0["pairs"][1] + q0["pairs"][2] + q0["pairs"][3])
        bg.add(qkv_flat(q1))
        # markers: att0-hp needs qkv0 pair hp; att1-hp needs qkv1 v+pair hp
        for hp in range(4):
            bg.drain_to(18 * hp)
            att_block(0, hp, 0, 4, "norm", bg,
                      2 * (3 - hp) + 16, mask_pe=True)
        for hp in range(4):
            bg.drain_to(108 + 18 * hp)
            att_block(1, hp, 0, 8, "norm", bg, 4 * (3 - hp),
                      mask_pe=True)
        bg.drain()
        fetch_xt(3)

        # P2: att(3) kb0..7 spilled (chunks 0,1 K/V + chunk-3 Q); qkv(3)
        # itself is the background, with per-hp markers for the Q pairs.
        new_yt(3)
        new_yt(2)
        q3 = qkv_ops(3, nc.vector)
        bg = Bg(qkv_flat(q3))
        for hp in range(4):
            bg.drain_to(36 + 18 * (hp + 1))
            att_block(3, hp, 0, 8, "spill", bg, 4 * (3 - hp))
        bg.drain()

        # P3: per head-pair, att(3) kb8..15 (resume + norm) then att(2)
        # kb0..7 spilled; qkv(2) + proj(0) + proj(1) as background.
        q2 = qkv_ops(2, nc.vector)
        bg = Bg(qkv_flat(q2))
        bg.add(proj_ops(0, nc.vector) + proj_ops(1, nc.vector))
        for hp in range(4):
            batches_after = (3 - hp) * 8
            bg.drain_to(36 + 18 * (hp + 1))
            att_block(3, hp, 8, 16, "final_spill", bg, batches_after + 4)
            att_block(2, hp, 0, 8, "spill", bg, batches_after)
        bg.drain()

        # P4: att(2) kb8..11 resumed + normalized (hp 3 first so the
        # projection's last-consumed head-pair is the last one normalized);
        # proj(3) as background.
        bg = Bg(proj_ops(3, nc.vector))
        for i, hp in enumerate((3, 0, 1, 2)):
            att_block(2, hp, 8, 12, "final_spill", bg, 2 * (3 - i),
                      mask_pe=True)
        bg.drain()

        for op in proj_ops(2, nc.vector, hp_order=(3, 0, 1, 2)):
            op()

    nc.compile()
    _CACHE["nc"] = nc
    return nc


def _masks():
    kk = np.arange(128)[:, None]
    qq = np.arange(512)[None, :]
    mask1 = np.stack([(qq >= m * 128 + kk) for m in range(4)])
    maskt_np = np.concatenate([mask1, mask1], axis=2).astype(
        ml_dtypes.bfloat16)
    maskb_np = np.zeros((5, 128, 1024), dtype=np.float32)
    maskb_np[0:4] = (maskt_np.astype(np.float32) - 1.0) * 240.0
    maskb_np[4, :, 0:128] = np.eye(128, dtype=np.float32)
    maskb_np = maskb_np.astype(ml_dtypes.bfloat16)
    # [m, 128, c] -> [128, m, c] for contiguous per-partition DMA
    return (np.ascontiguousarray(maskt_np.transpose(1, 0, 2)),
            np.ascontiguousarray(maskb_np.transpose(1, 0, 2)))


def _prep_core(x2d, W_attn, b_attn, W_proj, s, maskt_np, maskb_np):
    """Build one core's in_map; all tensors pre-transposed to
    [128 partitions, ...] contiguous layouts."""
    bf16 = ml_dtypes.bfloat16
    xT_c = np.ascontiguousarray(
        x2d.T.reshape(KT, 128, 4, 512).transpose(1, 2, 0, 3)).astype(bf16)
    wqk_c = np.concatenate([W_attn[:, s:s + HG],
                            W_attn[:, N + s:N + s + HG]], axis=1)
    wqk_c = np.ascontiguousarray(
        wqk_c.reshape(KT, 128, 2 * HG).transpose(1, 0, 2)).astype(bf16)
    wv_c = W_attn[:, 2 * N + s:2 * N + s + HG]
    wv_c = np.ascontiguousarray(
        wv_c.reshape(KT, 128, HG).transpose(1, 0, 2)).astype(bf16)
    wp_c = np.ascontiguousarray(
        W_proj[s:s + HG, :].reshape(4, 128, N).transpose(1, 0, 2)).astype(
        bf16)
    bqk_c = np.concatenate([b_attn[s:s + HG], b_attn[N + s:N + s + HG]])
    bqk_c = np.ascontiguousarray(
        bqk_c.reshape(8, 128, 1).transpose(1, 0, 2)).astype(np.float32)
    return {"xT": xT_c, "wqk": wqk_c, "wv": wv_c, "wp": wp_c,
            "bqk": bqk_c, "maskt": maskt_np, "maskb": maskb_np}


def kernel(x, W_attn, b_attn, W_proj, b_proj):
    global _last_results
    nc = _build()

    x = np.asarray(x, dtype=np.float32)
    W_attn = np.asarray(W_attn, dtype=np.float32)
    b_attn = np.asarray(b_attn, dtype=np.float32)
    W_proj = np.asarray(W_proj, dtype=np.float32)
    b_proj = np.asarray(b_proj, dtype=np.float32)

    maskt_np, maskb_np = _masks()
    in_maps = []
    for c in range(NCORES):
        b, hg = divmod(c, 2)
        in_maps.append(_prep_core(x[b], W_attn, b_attn, W_proj, hg * HG,
                                  maskt_np, maskb_np))

    res = run_bass_kernel_spmd(nc, in_maps, list(range(NCORES)))
    _last_results = res
    outs = [res.results[c]["out"] for c in range(NCORES)]
    # v-bias: softmax rows sum to 1, so att @ (xWv + bv) = att @ (xWv) + bv;
    # its projection (bv @ W_proj) plus b_proj are added here, exactly.
    bv = b_attn[2 * N:3 * N]
    extra = bv @ W_proj + b_proj
    y = np.stack([outs[2 * b] + outs[2 * b + 1] for b in range(B)])
    return (y + extra[None, None, :]).astype(np.float32)


# revision 41
# speedup vs baseline: 1.0321x; 1.0321x over previous
"""Multi-head causal attention (B=4, T=2048, N=1024, H=16) on 8 TRN2 NeuronCores.

Sharding: core c = (batch b = c//2, head-group hg = c%2). Each core computes
full-T causal attention for its 8 heads of its batch, plus the partial output
projection for its head rows. Host sums the two head-group partials per batch
and adds b_proj (and the v-bias contribution, which is exact because softmax
rows sum to 1).

v3 schedule — one fused stream balancing PE against the ACT engine (exp):
  - Work order: qkv(0); att(0) [qkv(1) bg]; att(1) [qkv(3) bg]; then per
    head-pair: att(3) full + att(2) kb0..7 spilled to SBUF [qkv(2), proj(0),
    proj(1) bg]; att(2) kb8..11 resumed + normalized [proj(3) bg]; proj(2).
    This spreads attention exp (ACT-bound, 2:1 vs its PE stream) across the
    qkv/proj filler so neither engine stalls long.
  - Scores per (head-pair, k-block) are two K=64 matmuls on PE row groups
    0/64 (run concurrently); emitted two k-blocks at a time so the PE
    64/128-row tiling mode switches half as often.
  - ACT runs ONLY exp: one [128,1024] activation per k-block spanning both
    heads' score banks. Bias-adds / drains / normalize are DVE+GPSIMD.
  - Projection in bf16 (weights + normalized y) to shorten the exposed tail.
PSUM: score-pair tile (2 banks) x2 + y accumulators x2 + qkv/proj rotation x2.
"""
import numpy as np
import ml_dtypes
from contextlib import ExitStack

import concourse.bass as bass
import concourse.mybir as mybir
from concourse import bacc
from concourse import bass_utils as _bu
from concourse.bass_utils import run_bass_kernel_spmd
from concourse.tile import TileContext



F32 = mybir.dt.float32
F32R = mybir.dt.float32r
BF16 = mybir.dt.bfloat16
F8 = mybir.dt.float8e4
DR = mybir.MatmulPerfMode.DoubleRow
AF = mybir.ActivationFunctionType

B, T, N, H = 4, 2048, 1024, 16
Dh = 64
HG = 512            # head-group width per core (8 heads x 64)
NCORES = 8
KT = N // 128       # 8 contraction tiles for qkv
NQC = T // 512      # 4 q-chunks
NVT = T // 128      # 16 v tiles / k blocks

_CACHE = {}
_last_results = None


def _build():
    if "nc" in _CACHE:
        return _CACHE["nc"]

    nc = bacc.Bacc("TRN2", target_bir_lowering=False)

    # all inputs pre-transposed host-side to [128 partitions, ...] so each
    # DMA descriptor covers the full per-partition extent (contiguous)
    xT = nc.declare_dram_parameter("xT", [128, 4, KT, 512], BF16,
                                   isOutput=False)
    wqk = nc.declare_dram_parameter("wqk", [128, KT, 2 * HG], BF16,
                                    isOutput=False)
    wv = nc.declare_dram_parameter("wv", [128, KT, HG], BF16, isOutput=False)
    wp = nc.declare_dram_parameter("wp", [128, 4, N], BF16, isOutput=False)
    bqk = nc.declare_dram_parameter("bqk", [128, 8, 1], F32, isOutput=False)
    maskt = nc.declare_dram_parameter("maskt", [128, 4, 1024], BF16,
                                      isOutput=False)
    # bias-form masks (0 keep / -240 masked) + identity in slice 4 cols 0:128
    maskb = nc.declare_dram_parameter("maskb", [128, 5, 1024], BF16,
                                      isOutput=False)
    out = nc.declare_dram_parameter("out", [T, N], F32, isOutput=True)

    with TileContext(nc) as tc, ExitStack() as outer:
        consts = outer.enter_context(tc.tile_pool(name="consts", bufs=1))
        resid = outer.enter_context(tc.tile_pool(name="resid", bufs=1))
        wpool = outer.enter_context(tc.tile_pool(name="wpool", bufs=1))
        xpool = outer.enter_context(tc.tile_pool(name="xpool", bufs=3))
        attTp = outer.enter_context(tc.tile_pool(name="attTp", bufs=5))
        smallp = outer.enter_context(tc.tile_pool(name="smallp", bufs=2))
        bcp = outer.enter_context(tc.tile_pool(name="bcp", bufs=2))
        spillp = outer.enter_context(tc.tile_pool(name="spillp", bufs=1))
        ytp = outer.enter_context(tc.tile_pool(name="ytp", bufs=3))
        outp = outer.enter_context(tc.tile_pool(name="outp", bufs=2))
        # PSUM pools: 2*2 + 2*1 + 2*1 = 8 banks
        sps = outer.enter_context(tc.tile_pool(name="sps", bufs=2,
                                               space="PSUM"))
        yps = outer.enter_context(tc.tile_pool(name="yps", bufs=2,
                                               space="PSUM"))
        gps = outer.enter_context(tc.tile_pool(name="gps", bufs=2,
                                               space="PSUM"))

        # ---- DMAs: one consolidated trigger per tensor group ----
        # (each DRAM view is transposed into [128, k, cols] tiles so a
        # single descriptor-rich DMA replaces 8 separate triggers)
        xt_chunks = [None] * 4

        def fetch_xt(tck):
            t = xpool.tile([128, KT, 512], BF16, name=f"xt{tck}", tag="xt")
            if tck == 0:
                nc.sync.dma_start(out=t[:, 0:4, :], in_=xT[:, 0, 0:4, :])
                nc.sync.dma_start(out=t[:, 4:8, :], in_=xT[:, 0, 4:8, :])
            else:
                nc.sync.dma_start(out=t, in_=xT[:, tck, :, :])
            xt_chunks[tck] = t

        # startup triggers split across both HWDGE queues (scalar queue is
        # exp-free until ~25us, so a few early triggers are harmless there)
        bqk_sb = consts.tile([128, 8, 1], F32, name="bqk_sb", tag="bqk")
        nc.sync.dma_start(out=bqk_sb, in_=bqk[:, :, :])
        fetch_xt(0)
        wv_sb = wpool.tile([128, KT, HG], BF16, name="wv_sb", tag="wv")
        nc.sync.dma_start(out=wv_sb, in_=wv[:, :, :])
        wqk_sb = wpool.tile([128, KT, 2 * HG], BF16, name="wqk_sb",
                            tag="wqk")
        nc.sync.dma_start(out=wqk_sb, in_=wqk[:, :, :])
        mt_all = consts.tile([128, 4, 1024], BF16, name="mt_all", tag="mt")
        nc.sync.dma_start(out=mt_all, in_=maskt[:, :, :])
        mb_all = consts.tile([128, 5, 1024], BF16, name="mb_all", tag="mb")
        nc.sync.dma_start(out=mb_all, in_=maskb[:, :, :])
        fetch_xt(1)
        wp_sb = wpool.tile([128, 4, N], BF16, name="wp_sb", tag="wp")
        nc.sync.dma_start(out=wp_sb, in_=wp[:, :, :])
        fetch_xt(2)

        wv_t = [wv_sb[:, k, :] for k in range(KT)]
        wqk_t = [wqk_sb[:, k, :] for k in range(KT)]
        wp_t = [wp_sb[:, j, :] for j in range(4)]
        bqk_sb = [bqk_sb[:, j, :] for j in range(8)]
        mt_sb = [mt_all[:, m, :] for m in range(4)]
        mb_sb = [mb_all[:, m, :] for m in range(4)]
        ident = mb_all[:, 4, 0:128]

        # residents: qT/kT per head-pair tile [128, T]; v per 128-row chunk,
        # bf16, layout [64 v-cols + ones] x 8 heads = 520 cols
        qT = [resid.tile([128, T], BF16, name=f"qT{j}", tag=f"qT{j}")
              for j in range(4)]
        kTt = [resid.tile([128, T], BF16, name=f"kT{j}", tag=f"kT{j}")
               for j in range(4)]
        vt = [resid.tile([128, 8, 65], BF16, name=f"v{m}", tag=f"v{m}")
              for m in range(NVT)]
        for m in range(NVT):
            nc.gpsimd.memset(vt[m][:, :, 64:65], 1.0)

        yt_all = {}     # qc -> [4] yt tiles (bf16, normalized yT)
        spill = {}      # (hp, h01) -> [65, 512] f32 SBUF partial (qc=2)

        class Bg:
            """FIFO of background PE micro-ops sprinkled between units."""
            def __init__(self, ops=()):
                self.ops = list(ops)
                self.done = 0

            def add(self, ops):
                self.ops.extend(ops)

            def pop(self, n=1):
                for _ in range(min(n, len(self.ops))):
                    self.ops.pop(0)()
                    self.done += 1

            def drain_to(self, k):
                while self.done < k and self.ops:
                    self.pop()

            def drain(self):
                while self.ops:
                    self.pop()

            def __len__(self):
                return len(self.ops)

        # ---- background micro-op machinery ----
        def qkv_ops(tck, drain_eng):
            """Returns {'v': [36 ops], 'pairs': [[18 ops] x 4 head-pairs]}."""
            c0 = tck * 512
            xt = xt_chunks[tck]

            def add_chain(ops, mk_mm, mk_drain, nk=KT):
                ps_ref = []
                for k in range(nk):
                    ops.append(mk_mm(k, ps_ref))
                ops.append(mk_drain(ps_ref))

            v_ops = []
            for mc in range(4):
                def mk_mm(k, ps_ref, mc=mc):
                    def go():
                        if k == 0:
                            ps_ref.append(gps.tile(
                                [128, HG], F32, name=f"v_ps{tck}_{mc}",
                                tag="gp"))
                        nc.tensor.matmul(
                            ps_ref[0],
                            xt[:, k, mc * 128:(mc + 1) * 128],
                            wv_sb[:, k, :],
                            start=(k == 0), stop=(k == KT - 1))
                    return go

                def mk_drain(ps_ref, mc=mc):
                    def go():
                        dst = vt[tck * 4 + mc]
                        src = ps_ref[0].rearrange("p (h c) -> p h c", c=64)
                        if drain_eng is nc.scalar:
                            nc.scalar.copy(dst[:, :, 0:64], src)
                        else:
                            drain_eng.tensor_copy(dst[:, :, 0:64], src)
                    return go
                add_chain(v_ops, mk_mm, mk_drain)

            pair_ops = []
            for hp in range(4):
                ops = []
                for jc in (hp, hp + 4):
                    def mk_mm(k, ps_ref, jc=jc):
                        def go():
                            if k == 0:
                                ps_ref.append(gps.tile(
                                    [128, 512], F32,
                                    name=f"qk_ps{tck}_{jc}", tag="gp"))
                            nc.tensor.matmul(
                                ps_ref[0],
                                wqk_sb[:, k, jc * 128:(jc + 1) * 128],
                                xt[:, k, :],
                                start=(k == 0), stop=(k == KT - 1))
                        return go

                    def mk_drain(ps_ref, jc=jc):
                        def go():
                            dst = (qT[jc] if jc < 4 else kTt[jc - 4])
                            if drain_eng is nc.scalar:
                                nc.scalar.add(dst[:, c0:c0 + 512],
                                              ps_ref[0], bqk_sb[jc])
                            else:
                                drain_eng.tensor_scalar_add(
                                    dst[:, c0:c0 + 512], ps_ref[0],
                                    bqk_sb[jc])
                        return go
                    add_chain(ops, mk_mm, mk_drain)
                pair_ops.append(ops)
            return {"v": v_ops, "pairs": pair_ops}

        def qkv_flat(q):
            ops = list(q["v"])
            for p in q["pairs"]:
                ops.extend(p)
            return ops

        def proj_ops(qc, drain_eng, hp_order=(0, 1, 2, 3)):
            qcol = qc * 512
            yt_sb = yt_all[qc]
            ops = []
            for qb in range(4):
                o_ref = []
                for nch in range(2):
                    ps_ref = []

                    def mk_mm(i, qb=qb, nch=nch, ps_ref=ps_ref,
                              o_ref=o_ref):
                        hp = hp_order[i]

                        def go():
                            if i == 0:
                                ps_ref.append(gps.tile(
                                    [128, 512], F32,
                                    name=f"p{qc}_{qb}_{nch}", tag="gp"))
                            if nch == 0 and i == 0:
                                o_ref.append(outp.tile(
                                    [128, N], F32, name=f"o{qc}_{qb}",
                                    tag="o"))
                            nc.tensor.matmul(
                                ps_ref[0],
                                yt_sb[hp][:, qb * 128:(qb + 1) * 128],
                                wp_t[hp][:, nch * 512:(nch + 1) * 512],
                                start=(i == 0), stop=(i == 3))
                        return go
                    for i in range(4):
                        ops.append(mk_mm(i, qb, nch, ps_ref, o_ref))

                    def mk_drain(qb=qb, nch=nch, ps_ref=ps_ref,
                                 o_ref=o_ref):
                        def go():
                            drain_eng.tensor_copy(
                                o_ref[0][:, nch * 512:(nch + 1) * 512],
                                ps_ref[0])
                            if nch == 1:
                                nc.sync.dma_start(
                                    out=out[qcol + qb * 128:
                                            qcol + (qb + 1) * 128, :],
                                    in_=o_ref[0])
                        return go
                    ops.append(mk_drain(qb, nch, ps_ref, o_ref))
            return ops

        # ---- attention block: one (qc, hp), k-blocks [kb_lo, kb_hi) ----
        # pops ceil(bg_share) background ops per pair-batch. mask_pe=True
        # applies the causal mask as an identity-matmul bias into the score
        # PSUM before exp (PE) instead of a post-exp DVE multiply.
        def att_block(qc, hp, kb_lo, kb_hi, mode, bg, batches_after,
                      mask_pe=False):
            qcol = qc * 512
            dlo = 4 * qc                   # first diagonal k-block
            y_ps = [yps.tile([65, 512], F32,
                             name=f"y{qc}_{hp}_{kb_lo}_{h01}", tag="y")
                    for h01 in range(2)]

            def emit_avs(at_l):
                for attT, kb in at_l:
                    for h01 in range(2):
                        nc.tensor.matmul(
                            y_ps[h01],
                            vt[kb][:, hp * 2 + h01, :],
                            attT[:, h01 * 512:(h01 + 1) * 512],
                            start=(kb == kb_lo),
                            stop=(kb == kb_hi - 1))

            pend = None
            nbatch = (kb_hi - kb_lo) // 2
            for bi in range(nbatch):
                kb0 = kb_lo + 2 * bi
                at_l = []
                for kb in (kb0, kb0 + 1):
                    diag = kb >= dlo
                    s_ps = sps.tile([128, 1024], F32,
                                    name=f"s{qc}_{hp}_{kb}", tag="s")
                    for h01 in range(2):
                        hb = h01 * 64
                        nc.tensor.matmul(
                            s_ps[:, h01 * 512:(h01 + 1) * 512],
                            kTt[hp][hb:hb + 64,
                                    kb * 128:(kb + 1) * 128],
                            qT[hp][hb:hb + 64, qcol:qcol + 512],
                            start=True, stop=not (diag and mask_pe))
                    if diag and mask_pe:
                        for h01 in range(2):
                            nc.tensor.matmul(
                                s_ps[:, h01 * 512:(h01 + 1) * 512],
                                ident,
                                mb_sb[kb - dlo][:, h01 * 512:
                                                (h01 + 1) * 512],
                                start=False, stop=True)
                    attT = attTp.tile([128, 1024], BF16,
                                      name=f"attT{qc}_{hp}_{kb}",
                                      tag="attT")
                    nc.scalar.activation(out=attT, in_=s_ps,
                                         func=AF.Exp, scale=0.125)
                    if diag and not mask_pe:
                        nc.vector.tensor_mul(attT, attT, mt_sb[kb - dlo])
                    at_l.append((attT, kb))
                if pend is not None:
                    emit_avs(pend)
                pend = at_l
                nleft = batches_after + (nbatch - 1 - bi)
                npop = -(-len(bg) // (nleft + 1))
                bg.pop(npop)
            emit_avs(pend)

            for h01 in range(2):
                hb = h01 * 64
                if mode == "spill":
                    sp = spillp.tile([65, 512], F32,
                                     name=f"sp{qc}_{hp}_{h01}",
                                     tag=f"sp{qc}{hp}{h01}")
                    nc.vector.tensor_copy(sp, y_ps[h01])
                    spill[(qc, hp, h01)] = sp
                    continue
                if mode == "final_spill":
                    # src lands in SBUF, so the copy/scale can use GPSIMD
                    sp = spill[(qc, hp, h01)]
                    nc.vector.tensor_add(sp, sp, y_ps[h01])
                    src = sp
                    copy_eng = mul_eng = nc.vector
                else:
                    src = y_ps[h01]
                    copy_eng = mul_eng = nc.vector
                srow = smallp.tile([1, 512], F32,
                                   name=f"srow{qc}_{hp}_{h01}", tag="srow")
                copy_eng.tensor_copy(srow, src[64:65, :])
                nc.vector.reciprocal_approx_fast(out=srow, in_=srow)
                bc = bcp.tile([64, 512], F32,
                              name=f"bc{qc}_{hp}_{h01}", tag="bc")
                nc.gpsimd.partition_broadcast(bc, srow)
                mul_eng.tensor_mul(yt_all[qc][hp][hb:hb + 64, :],
                                   src[0:64, :], bc)

        def new_yt(qc):
            yt_all[qc] = [ytp.tile([128, 512], BF16, name=f"yt{qc}_{hp}",
                                   tag=f"yt{hp}") for hp in range(4)]

        # ---- the fused stream ----
        # R01: qkv(0) v-chains + first qk pair direct, then att(0)+att(1)
        # with the rest of qkv(0) and qkv(1) as background.
        q0 = qkv_ops(0, nc.vector)
        for op in q0["v"] + q0["pairs"][0]:
            op()
        new_yt(0)
        new_yt(1)
        q1 = qkv_ops(1, nc.vector)
        bg = Bg(q0["pairs"][1] + q0["pairs"][2] + q0["pairs"][3])
        bg.add(qkv_flat(q1))
        # markers: att0-hp needs qkv0 pair hp; att1-hp needs qkv1 v+pair hp
        for hp in range(4):
            bg.drain_to(18 * hp)
            att_block(0, hp, 0, 4, "norm", bg,
                      2 * (3 - hp) + 16)
        for hp in range(4):
            bg.drain_to(108 + 18 * hp)
            att_block(1, hp, 0, 8, "norm", bg, 4 * (3 - hp))
        bg.drain()
        fetch_xt(3)

        # P2: att(3) kb0..7 spilled (chunks 0,1 K/V + chunk-3 Q); qkv(3)
        # itself is the background, with per-hp markers for the Q pairs.
        new_yt(3)
        new_yt(2)
        q3 = qkv_ops(3, nc.vector)
        bg = Bg(qkv_flat(q3))
        for hp in range(4):
            bg.drain_to(36 + 18 * (hp + 1))
            att_block(3, hp, 0, 8, "spill", bg, 4 * (3 - hp))
        bg.drain()

        # P3: per head-pair, att(3) kb8..15 (resume + norm) then att(2)
        # kb0..7 spilled; qkv(2) + proj(0) + proj(1) as background.
        q2 = qkv_ops(2, nc.vector)
        bg = Bg(qkv_flat(q2))
        bg.add(proj_ops(0, nc.vector) + proj_ops(1, nc.vector))
        for hp in range(4):
            batches_after = (3 - hp) * 8
            bg.drain_to(36 + 18 * (hp + 1))
            att_block(3, hp, 8, 16, "final_spill", bg, batches_after + 4)
            att_block(2, hp, 0, 8, "spill", bg, batches_after)
        bg.drain()

        # P4: att(2) kb8..11 resumed + normalized (hp 3 first so the
        # projection's last-consumed head-pair is the last one normalized);
        # proj(3) as background.
        bg = Bg(proj_ops(3, nc.vector))
        for i, hp in enumerate((3, 0, 1, 2)):
            att_block(2, hp, 8, 12, "final_spill", bg, 2 * (3 - i),
                      mask_pe=True)
        bg.drain()

        for op in proj_ops(2, nc.vector, hp_order=(3, 0, 1, 2)):
            op()

    nc.compile()
    _CACHE["nc"] = nc
    return nc


def _masks():
    kk = np.arange(128)[:, None]
    qq = np.arange(512)[None, :]
    mask1 = np.stack([(qq >= m * 128 + kk) for m in range(4)])
    maskt_np = np.concatenate([mask1, mask1], axis=2).astype(
        ml_dtypes.bfloat16)
    maskb_np = np.zeros((5, 128, 1024), dtype=np.float32)
    # scores on-device are (32q)·(32k); bias scaled to match
    maskb_np[0:4] = (maskt_np.astype(np.float32) - 1.0) * 240.0
    maskb_np[4, :, 0:128] = np.eye(128, dtype=np.float32)
    maskb_np = maskb_np.astype(ml_dtypes.bfloat16)
    # [m, 128, c] -> [128, m, c] for contiguous per-partition DMA
    return (np.ascontiguousarray(maskt_np.transpose(1, 0, 2)),
            np.ascontiguousarray(maskb_np.transpose(1, 0, 2)))


def _prep_core(x2d, W_attn, b_attn, W_proj, s, maskt_np, maskb_np):
    """Build one core's in_map; all tensors pre-transposed to
    [128 partitions, ...] contiguous layouts."""
    bf16 = ml_dtypes.bfloat16
    xT_c = np.ascontiguousarray(
        x2d.T.reshape(KT, 128, 4, 512).transpose(1, 2, 0, 3)).astype(bf16)
    wqk_c = np.concatenate([W_attn[:, s:s + HG],
                            W_attn[:, N + s:N + s + HG]], axis=1)
    wqk_c = np.ascontiguousarray(
        wqk_c.reshape(KT, 128, 2 * HG).transpose(1, 0, 2)).astype(bf16)
    wv_c = W_attn[:, 2 * N + s:2 * N + s + HG]
    wv_c = np.ascontiguousarray(
        wv_c.reshape(KT, 128, HG).transpose(1, 0, 2)).astype(bf16)
    wp_c = np.ascontiguousarray(
        W_proj[s:s + HG, :].reshape(4, 128, N).transpose(1, 0, 2)).astype(
        bf16)
    bqk_c = np.concatenate([b_attn[s:s + HG], b_attn[N + s:N + s + HG]])
    bqk_c = np.ascontiguousarray(
        bqk_c.reshape(8, 128, 1).transpose(1, 0, 2)).astype(np.float32)
    return {"xT": xT_c, "wqk": wqk_c, "wv": wv_c, "wp": wp_c,
            "bqk": bqk_c, "maskt": maskt_np, "maskb": maskb_np}


def kernel(x, W_attn, b_attn, W_proj, b_proj):
    global _last_results
    nc = _build()

    x = np.asarray(x, dtype=np.float32)
    W_attn = np.asarray(W_attn, dtype=np.float32)
    b_attn = np.asarray(b_attn, dtype=np.float32)
    W_proj = np.asarray(W_proj, dtype=np.float32)
    b_proj = np.asarray(b_proj, dtype=np.float32)

    maskt_np, maskb_np = _masks()
    in_maps = []
    for c in range(NCORES):
        b, hg = divmod(c, 2)
        in_maps.append(_prep_core(x[b], W_attn, b_attn, W_proj, hg * HG,
                                  maskt_np, maskb_np))

    res = run_bass_kernel_spmd(nc, in_maps, list(range(NCORES)))
    _last_results = res
    outs = [res.results[c]["out"] for c in range(NCORES)]
    # v-bias: softmax rows sum to 1, so att @ (xWv + bv) = att @ (xWv) + bv;
    # its projection (bv @ W_proj) plus b_proj are added here, exactly.
    bv = b_attn[2 * N:3 * N]
    extra = bv @ W_proj + b_proj
    y = np.stack([outs[2 * b] + outs[2 * b + 1] for b in range(B)])
    return (y + extra[None, None, :]).astype(np.float32)
